# revision 1
# baseline (speedup 1.0000x reference)
"""Trainium2 Bass kernel for the BGNN (3-layer GCN x 2 branches + mean-pool + MLP).

Contract: kernel(**inputs) takes FULL numpy inputs (keys as in
reference.setup_inputs()) and returns the FULL [G, 2] float32 output.
Internally: shards nodes+edges across 8 NeuronCores (dst-sharding),
gathers remote source features via dma_gather from a replicated
(AllGather'd) bf16 feature table, reduces messages with one-hot PE
matmuls, and evaluates the pooled MLP redundantly on every core.
"""
import sys

sys.path.insert(0, "/opt/trn_rl_repo")

import numpy as np
import ml_dtypes

import concourse.bacc as bacc
import concourse.bass as bass
import concourse.mybir as mybir
import concourse.tile as tile
from concourse.bass_utils import run_bass_kernel_spmd

P = 128
NCORE = 8
G = 64               # graphs per batch (fixed by the problem)
WG = 7               # windows per gather group
CHUNK_ROWS = 25088   # table rows addressable per int16 index chunk (<=32767)

last_results = None  # set by _run for test harness introspection


def _ceil_to(x, m):
    return (x + m - 1) // m * m


def _prep_branch(edge_index, batch, n, npad, sh, nw, ng, nchunk, cap=None):
    """Bucket edges by (dst-core, dst-window-group, src-chunk, window); build
    the padded int16 gather-index array, bf16 window-local dst array, dis,
    batch-local tiles and inverse counts."""
    src = edge_index[0].astype(np.int64)
    dst = edge_index[1].astype(np.int64)
    e = src.shape[0]

    deg = np.bincount(dst, minlength=n).astype(np.float32) + 1.0
    dis = deg ** -0.5                                   # [n]

    core = dst // sh
    win = (dst % sh) // P
    chunk = src // CHUNK_ROWS
    grp = win // WG
    win_in = win % WG
    key = ((core * ng + grp) * nchunk + chunk) * WG + win_in
    nbuckets = NCORE * ng * nchunk * WG
    order = np.argsort(key, kind="stable")
    key_s = key[order]
    counts = np.bincount(key_s, minlength=nbuckets)
    need = max(int(_ceil_to(counts.max(), P)), P)
    if cap is None:
        cap = need
    assert cap >= need
    bpb = cap // P

    starts = np.zeros(nbuckets, np.int64)
    np.cumsum(counts[:-1], out=starts[1:])
    rank = np.arange(e, dtype=np.int64) - starts[key_s]
    slot = key_s * cap + rank

    idx_flat = np.zeros(nbuckets * cap, np.int16)
    dl_flat = np.full(nbuckets * cap, -1.0, np.float32)
    idx_flat[slot] = (src[order] - chunk[order] * CHUNK_ROWS).astype(np.int16)
    dl_flat[slot] = (dst[order] % P).astype(np.float32)
    # pad idx slots repeat the bucket's last valid index (duplicate fetch, no effect)
    idx_mat = idx_flat.reshape(nbuckets, cap)
    has = counts > 0
    lastv = np.zeros(nbuckets, np.int16)
    lastv[has] = idx_mat[has, np.minimum(counts[has] - 1, cap - 1)]
    pad_mask = np.arange(cap)[None, :] >= counts[:, None]
    idx_mat[pad_mask] = np.broadcast_to(lastv[:, None], idx_mat.shape)[pad_mask]

    # per-core wrapped layouts
    idx_pc = idx_mat.reshape(NCORE, ng * nchunk * WG * cap)
    idx_w = idx_pc.reshape(NCORE, -1, 16).transpose(0, 2, 1)      # [NCORE,16,cols]
    idx_w = np.ascontiguousarray(np.tile(idx_w, (1, 8, 1)))       # [NCORE,128,cols]
    dl_pc = dl_flat.reshape(NCORE, ng * nchunk * WG * bpb, P)
    dl_w = np.ascontiguousarray(dl_pc.transpose(0, 2, 1)).astype(ml_dtypes.bfloat16)

    dis_pad = np.ones(npad, np.float32)
    dis_pad[:n] = dis
    dis_t = np.ascontiguousarray(dis_pad.reshape(NCORE, nw, P).transpose(0, 2, 1))
    bl_pad = np.full(npad, -1.0, np.float32)
    bl_pad[:n] = batch.astype(np.float32)
    bl_t = np.ascontiguousarray(
        bl_pad.reshape(NCORE, nw, P).transpose(0, 2, 1)).astype(ml_dtypes.bfloat16)

    cnt = np.bincount(batch.astype(np.int64), minlength=G).astype(np.float32)
    inv_cnt = (1.0 / np.maximum(cnt, 1.0)).reshape(G, 1)

    return dict(idx=idx_w, dl=dl_w, dis=dis_t, bl=bl_t, inv_cnt=inv_cnt,
                cap=cap, need=need, bpb=bpb)


def _build_program(npad, sh, nw, ng, nchunk, bpb, d_feat, msg_bufs=2):
    nc = bacc.Bacc()
    bf16 = mybir.dt.bfloat16
    f32 = mybir.dt.float32
    i16 = mybir.dt.int16
    cap = bpb * P
    nblk = ng * nchunk * WG * bpb
    idx_cols = ng * nchunk * WG * cap // 16
    gcols = WG * cap // 16
    gblk = WG * bpb
    DIMS = [d_feat, 32, 16, 8]

    # ---------------- parameters ----------------
    prm = {}
    for b in (0, 1):
        prm[f"x{b}"] = nc.declare_dram_parameter(f"x{b}", [sh, d_feat], f32, isOutput=False)
        prm[f"idx{b}"] = nc.declare_dram_parameter(f"idx{b}", [P, idx_cols], i16, isOutput=False)
        prm[f"dl{b}"] = nc.declare_dram_parameter(f"dl{b}", [P, nblk], bf16, isOutput=False)
        prm[f"dis{b}"] = nc.declare_dram_parameter(f"dis{b}", [P, nw], f32, isOutput=False)
        prm[f"bl{b}"] = nc.declare_dram_parameter(f"bl{b}", [P, nw], bf16, isOutput=False)
        prm[f"ic{b}"] = nc.declare_dram_parameter(f"ic{b}", [G, 1], f32, isOutput=False)
    ident_in = nc.declare_dram_parameter("ident", [P, P], f32, isOutput=False)
    iota128_in = nc.declare_dram_parameter("iota128", [P, P], bf16, isOutput=False)
    iota64_in = nc.declare_dram_parameter("iota64", [P, G], bf16, isOutput=False)
    W_in = [nc.declare_dram_parameter(f"W{l+1}", [DIMS[l], DIMS[l+1]], f32, isOutput=False) for l in range(3)]
    B_in = [nc.declare_dram_parameter(f"b{l+1}r", [P, DIMS[l+1]], f32, isOutput=False) for l in range(3)]
    mW1_in = nc.declare_dram_parameter("mW1", [16, 8], f32, isOutput=False)
    mb1_in = nc.declare_dram_parameter("mb1r", [G, 8], f32, isOutput=False)
    mW2_in = nc.declare_dram_parameter("mW2", [8, 2], f32, isOutput=False)
    mb2_in = nc.declare_dram_parameter("mb2r", [G, 2], f32, isOutput=False)
    out_p = nc.declare_dram_parameter("out", [G, 2], f32, isOutput=True)

    # ---------------- internal DRAM ----------------
    tabfull = nc.dram_tensor("tabfull", [npad, P], bf16)
    agin = [nc.dram_tensor(f"agin{l}", [sh, DIMS[l + 1]], bf16) for l in range(3)]
    agfull = [nc.dram_tensor(f"agfull{l}", [npad, DIMS[l + 1]], bf16) for l in range(3)]
    pool_in = nc.dram_tensor("pool_in", [G, 8], f32)
    pool_out = nc.dram_tensor("pool_out", [G, 8], f32)

    with tile.TileContext(nc) as tc:
        with (
            tc.tile_pool(name="const", bufs=1) as cp,
            tc.tile_pool(name="resident", bufs=1) as rp,
            tc.tile_pool(name="stream", bufs=msg_bufs) as spool,
            tc.tile_pool(name="small", bufs=3) as sm,
        ):
            # ---- constants ----
            ident = cp.tile([P, P], f32)
            nc.sync.dma_start(out=ident[:, :], in_=ident_in[:, :])
            iota128 = cp.tile([P, P], bf16)
            nc.sync.dma_start(out=iota128[:, :], in_=iota128_in[:, :])
            iota64 = cp.tile([P, G], bf16)
            nc.sync.dma_start(out=iota64[:, :], in_=iota64_in[:, :])
            Ws, Bs = [], []
            for l in range(3):
                w = cp.tile([DIMS[l], DIMS[l + 1]], f32, tag=f"w{l}")
                nc.sync.dma_start(out=w[:, :], in_=W_in[l][:, :])
                Ws.append(w)
                bb = cp.tile([P, DIMS[l + 1]], f32, tag=f"b{l}")
                nc.sync.dma_start(out=bb[:, :], in_=B_in[l][:, :])
                Bs.append(bb)
            mW1 = cp.tile([16, 8], f32)
            nc.sync.dma_start(out=mW1[:, :], in_=mW1_in[:, :])
            mb1 = cp.tile([G, 8], f32)
            nc.sync.dma_start(out=mb1[:, :], in_=mb1_in[:, :])
            mW2 = cp.tile([8, 2], f32)
            nc.sync.dma_start(out=mW2[:, :], in_=mW2_in[:, :])
            mb2 = cp.tile([G, 2], f32)
            nc.sync.dma_start(out=mb2[:, :], in_=mb2_in[:, :])

            pooled_cat = rp.tile([G, 16], f32)

            for br in (0, 1):
                dl_t = rp.tile([P, nblk], bf16, tag="dl")
                nc.sync.dma_start(out=dl_t[:, :], in_=prm[f"dl{br}"][:, :])
                dis_t = rp.tile([P, nw], f32, tag="dis")
                nc.sync.dma_start(out=dis_t[:, :], in_=prm[f"dis{br}"][:, :])
                bl_t = rp.tile([P, nw], bf16, tag="bl")
                nc.sync.dma_start(out=bl_t[:, :], in_=prm[f"bl{br}"][:, :])

                hself = rp.tile([P, nw, 32], f32, tag="hself")
                act = rp.tile([P, nw, 32], f32, tag="act")

                for l in range(3):
                    din, dout = DIMS[l], DIMS[l + 1]
                    # ======== table build: tab = dis * (act_in @ W) ========
                    pt_ctx = tc.tile_pool(name=f"pt_{br}_{l}", bufs=2, space="PSUM")
                    pt = pt_ctx.__enter__()
                    for t in range(nw):
                        if l == 0:
                            a_t = sm.tile([P, d_feat], f32, tag="a_in")
                            nc.sync.dma_start(out=a_t[:, :],
                                              in_=prm[f"x{br}"][t * P:(t + 1) * P, :])
                            a_ap = a_t[:, :]
                        else:
                            a_ap = act[:, t, :din]
                        aT_p = pt.tile([din, P], f32, tag="aT_p", space="PSUM")
                        nc.tensor.transpose(out=aT_p[:, :], in_=a_ap, identity=ident[:, :])
                        aT_s = sm.tile([din, P], f32, tag="aT_s")
                        nc.vector.tensor_copy(out=aT_s[:, :], in_=aT_p[:, :])
                        h_p = pt.tile([P, dout], f32, tag="h_p", space="PSUM")
                        nc.tensor.matmul(out=h_p[:, :], lhsT=aT_s[:, :], rhs=Ws[l][:, :],
                                         start=True, stop=True)
                        nc.vector.tensor_scalar_mul(out=hself[:, t, :dout], in0=h_p[:, :],
                                                    scalar1=dis_t[:, t:t + 1])
                        stage = sm.tile([P, dout], bf16, tag="stage")
                        nc.vector.tensor_copy(out=stage[:, :], in_=hself[:, t, :dout])
                        nc.sync.dma_start(out=agin[l][t * P:(t + 1) * P, :], in_=stage[:, :])
                    pt_ctx.__exit__(None, None, None)

                    # ======== AllGather table + expand into tabfull ========
                    nc.gpsimd.collective_compute(
                        "AllGather", mybir.AluOpType.bypass,
                        replica_groups=[list(range(NCORE))],
                        ins=[agin[l][:, :]], outs=[agfull[l][:, :]],
                    )
                    half = npad // 2
                    nc.sync.dma_start(out=tabfull[0:half, 0:dout],
                                      in_=agfull[l][0:half, :])
                    nc.sync.dma_start(out=tabfull[half:npad, 0:dout],
                                      in_=agfull[l][half:npad, :])

                    # ======== gather + one-hot reduce ========
                    pg_ctx = tc.tile_pool(name=f"pg_{br}_{l}", bufs=1, space="PSUM")
                    pg = pg_ctx.__enter__()
                    for g in range(ng):
                        aggs = [pg.tile([P, 32], f32, tag=f"agg{wi}",
                                        name=f"agg_{br}_{l}_{g}_{wi}", space="PSUM")
                                for wi in range(WG)]
                        for c in range(nchunk):
                            gi = g * nchunk + c
                            idx_t = spool.tile([P, gcols], i16, tag="idx")
                            nc.sync.dma_start(out=idx_t[:, :],
                                              in_=prm[f"idx{br}"][:, gi * gcols:(gi + 1) * gcols])
                            msgs = spool.tile([P, gblk, P], bf16, tag="msgs")
                            nc.gpsimd.dma_gather(
                                out_ap=msgs[:, :, :],
                                in_ap=tabfull[c * CHUNK_ROWS:min((c + 1) * CHUNK_ROWS, npad), :],
                                idxs_ap=idx_t[:, :], num_idxs=WG * cap,
                                num_idxs_reg=WG * cap, elem_size=P,
                                single_packet=False,
                            )
                            sd = spool.tile([P, gblk, P], bf16, tag="sd")
                            nc.vector.tensor_tensor(
                                out=sd[:, :, :],
                                in0=dl_t[:, gi * gblk:(gi + 1) * gblk, None].to_broadcast([P, gblk, P]),
                                in1=iota128[:, None, :].to_broadcast([P, gblk, P]),
                                op=mybir.AluOpType.is_equal,
                            )
                            for wi in range(WG):
                                for b2 in range(bpb):
                                    blk = wi * bpb + b2
                                    nc.tensor.matmul(
                                        out=aggs[wi][:, :dout],
                                        lhsT=sd[:, blk, :], rhs=msgs[:, blk, :dout],
                                        start=(c == 0 and b2 == 0),
                                        stop=(c == nchunk - 1 and b2 == bpb - 1),
                                    )
                        for wi in range(WG):
                            w = g * WG + wi
                            t1 = sm.tile([P, dout], f32, tag="post1")
                            nc.vector.tensor_add(out=t1[:, :], in0=aggs[wi][:, :dout],
                                                 in1=hself[:, w, :dout])
                            nc.vector.tensor_scalar(
                                out=act[:, w, :dout], in0=t1[:, :],
                                scalar1=dis_t[:, w:w + 1], scalar2=None,
                                op0=mybir.AluOpType.mult,
                            )
                            nc.vector.tensor_add(out=act[:, w, :dout],
                                                 in0=act[:, w, :dout], in1=Bs[l][:, :])
                            if l < 2:
                                nc.vector.tensor_scalar_max(out=act[:, w, :dout],
                                                            in0=act[:, w, :dout],
                                                            scalar1=0.0)
                    pg_ctx.__exit__(None, None, None)

                # ======== mean-pool branch -> pooled_cat[:, br*8 : br*8+8] ========
                pp_ctx = tc.tile_pool(name=f"pp_{br}", bufs=1, space="PSUM")
                pp = pp_ctx.__enter__()
                pool_p = pp.tile([G, 8], f32, tag="h_p", name="pool_p", space="PSUM")
                for t in range(nw):
                    oh = sm.tile([P, G], bf16, tag="pool_oh")
                    nc.vector.tensor_tensor(
                        out=oh[:, :],
                        in0=bl_t[:, t:t + 1].to_broadcast([P, G]),
                        in1=iota64[:, :], op=mybir.AluOpType.is_equal,
                    )
                    a_bf = sm.tile([P, 8], bf16, tag="pool_in_bf")
                    nc.vector.tensor_copy(out=a_bf[:, :], in_=act[:, t, :8])
                    nc.tensor.matmul(out=pool_p[:, :], lhsT=oh[:, :], rhs=a_bf[:, :],
                                     start=(t == 0), stop=(t == nw - 1))
                pool_s = sm.tile([G, 8], f32, tag="pool_s")
                nc.vector.tensor_copy(out=pool_s[:, :], in_=pool_p[:, :])
                nc.sync.dma_start(out=pool_in[:, :], in_=pool_s[:, :])
                nc.gpsimd.collective_compute(
                    "AllReduce", mybir.AluOpType.add,
                    replica_groups=[list(range(NCORE))],
                    ins=[pool_in[:, :]], outs=[pool_out[:, :]],
                )
                pool_r = sm.tile([G, 8], f32, tag="pool_r")
                nc.sync.dma_start(out=pool_r[:, :], in_=pool_out[:, :])
                ic_t = sm.tile([G, 1], f32, tag="ic")
                nc.sync.dma_start(out=ic_t[:, :], in_=prm[f"ic{br}"][:, :])
                nc.vector.tensor_scalar_mul(out=pooled_cat[:, br * 8:br * 8 + 8],
                                            in0=pool_r[:, :], scalar1=ic_t[:, :])
                pp_ctx.__exit__(None, None, None)

            # ======== MLP: relu(cat @ mW1 + mb1) @ mW2 + mb2 ========
            pm_ctx = tc.tile_pool(name="pm", bufs=1, space="PSUM")
            pm = pm_ctx.__enter__()
            pcT_p = pm.tile([16, G], f32, tag="aT_p", name="pcT_p", space="PSUM")
            nc.tensor.transpose(out=pcT_p[:, :], in_=pooled_cat[:, :], identity=ident[:G, :G])
            pcT_s = sm.tile([16, G], f32, tag="pcT_s")
            nc.vector.tensor_copy(out=pcT_s[:, :], in_=pcT_p[:, :])
            m1_p = pm.tile([G, 8], f32, tag="h_p", name="m1_p", space="PSUM")
            nc.tensor.matmul(out=m1_p[:, :], lhsT=pcT_s[:, :], rhs=mW1[:, :],
                             start=True, stop=True)
            m1_s = sm.tile([G, 8], f32, tag="m1s")
            nc.vector.tensor_add(out=m1_s[:, :], in0=m1_p[:, :], in1=mb1[:, :])
            nc.vector.tensor_scalar_max(out=m1_s[:, :], in0=m1_s[:, :], scalar1=0.0)
            m1T_p = pm.tile([8, G], f32, tag="aT_p2", name="m1T_p", space="PSUM")
            nc.tensor.transpose(out=m1T_p[:, :], in_=m1_s[:, :], identity=ident[:G, :G])
            m1T_s = sm.tile([8, G], f32, tag="m1Ts")
            nc.vector.tensor_copy(out=m1T_s[:, :], in_=m1T_p[:, :])
            m2_p = pm.tile([G, 2], f32, tag="h_p2", name="m2_p", space="PSUM")
            nc.tensor.matmul(out=m2_p[:, :], lhsT=m1T_s[:, :], rhs=mW2[:, :],
                             start=True, stop=True)
            m2_s = sm.tile([G, 2], f32, tag="m2s")
            nc.vector.tensor_add(out=m2_s[:, :], in0=m2_p[:, :], in1=mb2[:, :])
            nc.sync.dma_start(out=out_p[:, :], in_=m2_s[:, :])
            pm_ctx.__exit__(None, None, None)

    nc.compile()
    return nc


def _run(inputs, trace=False, msg_bufs=2):
    global last_results
    x0 = np.asarray(inputs["x0"], np.float32)
    x1 = np.asarray(inputs["x1"], np.float32)
    n, d_feat = x0.shape
    ei0 = np.asarray(inputs["edge_index0"])
    ei1 = np.asarray(inputs["edge_index1"])
    b0 = np.asarray(inputs["batch0"])
    b1 = np.asarray(inputs["batch1"])

    per_core = (n + NCORE - 1) // NCORE
    sh = _ceil_to(per_core, P * WG)       # windows per core divisible by WG
    npad = sh * NCORE
    nw = sh // P
    ng = nw // WG
    nchunk = max(1, (npad + CHUNK_ROWS - 1) // CHUNK_ROWS)

    pb0 = _prep_branch(ei0, b0, n, npad, sh, nw, ng, nchunk)
    pb1 = _prep_branch(ei1, b1, n, npad, sh, nw, ng, nchunk)
    cap = max(pb0["cap"], pb1["cap"])
    if pb0["cap"] != cap:
        pb0 = _prep_branch(ei0, b0, n, npad, sh, nw, ng, nchunk, cap=cap)
    if pb1["cap"] != cap:
        pb1 = _prep_branch(ei1, b1, n, npad, sh, nw, ng, nchunk, cap=cap)
    bpb = cap // P

    xp = []
    for x in (x0, x1):
        t = np.zeros((npad, d_feat), np.float32)
        t[:n] = x
        xp.append(t.reshape(NCORE, sh, d_feat))

    ident = np.eye(P, dtype=np.float32)
    iota128 = np.broadcast_to(np.arange(P, dtype=np.float32), (P, P)).astype(ml_dtypes.bfloat16)
    iota64 = np.broadcast_to(np.arange(G, dtype=np.float32), (P, G)).astype(ml_dtypes.bfloat16)

    def wgt(name):
        return np.asarray(inputs[name], np.float32)

    common = dict(
        ident=ident, iota128=np.ascontiguousarray(iota128),
        iota64=np.ascontiguousarray(iota64),
        W1=wgt("W1"), W2=wgt("W2"), W3=wgt("W3"),
        b1r=np.broadcast_to(wgt("b1"), (P, 32)).copy(),
        b2r=np.broadcast_to(wgt("b2"), (P, 16)).copy(),
        b3r=np.broadcast_to(wgt("b3"), (P, 8)).copy(),
        mW1=wgt("mW1"), mb1r=np.broadcast_to(wgt("mb1"), (G, 8)).copy(),
        mW2=wgt("mW2"), mb2r=np.broadcast_to(wgt("mb2"), (G, 2)).copy(),
        ic0=pb0["inv_cnt"], ic1=pb1["inv_cnt"],
    )
    in_maps = []
    for c in range(NCORE):
        m = dict(common)
        m["x0"] = np.ascontiguousarray(xp[0][c])
        m["x1"] = np.ascontiguousarray(xp[1][c])
        for name, pb in (("0", pb0), ("1", pb1)):
            m[f"idx{name}"] = pb["idx"][c]
            m[f"dl{name}"] = pb["dl"][c]
            m[f"dis{name}"] = pb["dis"][c]
            m[f"bl{name}"] = pb["bl"][c]
        in_maps.append(m)

    nc = _build_program(npad, sh, nw, ng, nchunk, bpb, d_feat, msg_bufs=msg_bufs)
    res = run_bass_kernel_spmd(nc, in_maps, list(range(NCORE)), trace=trace)
    last_results = res
    return np.asarray(res.results[0]["out"], np.float32)


def kernel(**inputs):
    return _run(inputs, trace=False)



# revision 7
# speedup vs baseline: 3.6841x; 3.6841x over previous
"""Trainium2 Bass kernel for the BGNN (3-layer GCN x 2 branches + mean-pool + MLP).

v2 design (ap_gather-based):
  - Nodes dst-sharded across 8 cores (SH=12544/core). Per branch, edges split
    into 8 gather streams by src%8; each GpSimd Q7 core gathers its stream's
    source features from an SBUF-resident replicated table via ap_gather
    (features transposed onto partitions, 8-node column packing).
  - L1 table bf16 feat-pairs in u32 units ([128, NT, 2] bf16); L2 table f32
    [128, NT]. Tables device-built per band, AllGathered, reloaded packed.
  - Per 128-edge-slot block: TensorE strided-plane transposes flip [feat,edge]
    to [edge,feat]; dl-vs-iota one-hot matmuls scatter into per-window PSUM
    accumulators (bank-first start, bank-last stop).
  - Layer 3 + mean-pool folded into host-precomputed structural matrix
    Dt[s,g]: pool = ((Dt^T @ act2)/cnt) @ W3 + b3.  MLP replicated per core.
  - Block->window schedule baked into the SPMD program: per-(stream,window)
    run capacities common across cores (max over cores, padded to 32);
    window-straddling blocks use multiple masked one-hot fragments.
"""
import sys

sys.path.insert(0, "/opt/trn_rl_repo")

import numpy as np
import ml_dtypes

import concourse.bacc as bacc
import concourse.bass as bass
import concourse.mybir as mybir
import concourse.tile as tile
from concourse.bass_utils import run_bass_kernel_spmd

P = 128
NCORE = 8
G = 64
N = 100000
SH = 12544
NPAD = SH * NCORE
NW = SH // P                # 98
NT = NPAD // 8              # 12544
CH = SH // 8                # 1568
KI = 4096
PI = np.concatenate([np.arange(0, 32, 2), np.arange(1, 32, 2)])

bfloat16 = ml_dtypes.bfloat16
last_results = None


def _pad_to(x, m):
    return (x + m - 1) // m * m


# --------------------------------------------------------------------------
# host prep
# --------------------------------------------------------------------------

def _prep_branch(ei, batch):
    src = ei[0].astype(np.int64)
    dst = ei[1].astype(np.int64)
    deg = np.bincount(dst, minlength=N).astype(np.float32) + 1.0
    dis = np.ones(NPAD, np.float32)
    dis[:N] = deg ** -0.5

    bpad = np.zeros(NPAD, np.int64)
    bpad[:N] = batch.astype(np.int64)
    flat = np.bincount(src * G + bpad[dst], weights=dis[dst].astype(np.float64),
                      minlength=NPAD * G)
    Dt = flat.reshape(NPAD, G).astype(np.float32)
    Dt *= dis[:, None]
    Dt[np.arange(N), bpad[:N]] += dis[:N] ** 2
    cnt = np.bincount(batch.astype(np.int64), minlength=G).astype(np.float32)
    inv_cnt = (1.0 / np.maximum(cnt, 1.0)).reshape(G, 1)

    core = dst // SH
    loc_s = src % SH
    stream = loc_s % 8
    gidx = (src // SH) * CH + loc_s // 8
    win = (dst % SH) // P
    dlv = (dst % P).astype(np.float32)

    key = (core * 8 + stream) * NW + win
    counts = np.bincount(key, minlength=NCORE * 8 * NW).reshape(NCORE, 8, NW)
    caps = _pad_to(counts.max(axis=0), 32)
    caps[0] = np.maximum(caps[0], 32)
    L = int(_pad_to(caps.sum(axis=1).max(), KI // 8 if False else P))
    NB = L // P

    starts = np.zeros((8, NW), np.int64)
    for q in range(8):
        np.cumsum(caps[q][:-1], out=starts[q][1:])

    # common block schedule
    frag_win = [[] for _ in range(NB * 8)]
    for q in range(8):
        for w in range(NW):
            s0, s1 = starts[q][w], starts[q][w] + caps[q][w]
            for k in range(s0 // P, min((s1 + P - 1) // P, NB)):
                frag_win[k * 8 + q].append(w)
    dlcols = []
    sched = []
    for t in range(NB * 8):
        kk, q = t // 8, t % 8
        ent = []
        for w in frag_win[t]:
            col = len(dlcols)
            s0, s1 = starts[q][w], starts[q][w] + caps[q][w]
            lo, hi = max(s0, kk * P), min(s1, (kk + 1) * P)
            dlcols.append((t, q, w, lo, hi))
            ent.append((col, w))
        sched.append(ent)
    TB = _pad_to(len(dlcols), 32)

    order = np.lexsort((win, stream, core))
    gidx_o, dl_o, win_o = gidx[order], dlv[order], win[order]
    core_o, stream_o = core[order], stream[order]
    cbounds = np.searchsorted(core_o, np.arange(NCORE + 1))
    per_core = []
    for c in range(NCORE):
        lo, hi = cbounds[c], cbounds[c + 1]
        gq, gw = stream_o[lo:hi], win_o[lo:hi]
        gi, gd = gidx_o[lo:hi], dl_o[lo:hi]
        idx_arr = np.zeros((8, L), np.int64)
        dl_full = np.full((8, NB * P), -1.0, np.float32)
        qb = np.searchsorted(gq, np.arange(9))
        for q in range(8):
            ql, qh = qb[q], qb[q + 1]
            wq, iq, dq = gw[ql:qh], gi[ql:qh], gd[ql:qh]
            wcnt = np.bincount(wq, minlength=NW)
            wstart = np.zeros(NW, np.int64)
            np.cumsum(wcnt[:-1], out=wstart[1:])
            pos = starts[q][wq] + (np.arange(qh - ql) - wstart[wq])
            idx_arr[q, pos] = iq
            dl_full[q, pos] = dq
        idx_tile = np.zeros((P, L // 16), np.int16)
        for g in range(8):
            idx_tile[16 * g:16 * g + 16, :] = \
                idx_arr[g].astype(np.int16).reshape(L // 16, 16).T
        dl_tile = np.full((P, TB), -1.0, np.float32)
        for col, (t, q, w, flo, fhi) in enumerate(dlcols):
            kk = t // 8
            seg = dl_full[q, kk * P:(kk + 1) * P].copy()
            mask = np.zeros(P, bool)
            mask[flo - kk * P:fhi - kk * P] = True
            seg[~mask] = -1.0
            dl_tile[:, col] = seg
        dis_t = np.ascontiguousarray(dis[c * SH:(c + 1) * SH].reshape(NW, P).T)
        disP = np.ascontiguousarray(dis[c * SH:(c + 1) * SH].reshape(CH, 8).T)
        DtT = np.ascontiguousarray(
            Dt[c * SH:(c + 1) * SH].reshape(NW, P, G).transpose(1, 0, 2)
            .reshape(P, NW * G)).astype(bfloat16)
        per_core.append(dict(idx=idx_tile, dl=dl_tile.astype(bfloat16),
                             dist=dis_t, dist2=dis_t * dis_t, disP=disP,
                             DtT=DtT))
    return dict(sched=sched, TB=TB, NB=NB, L=L, inv_cnt=inv_cnt,
                per_core=per_core, dis=dis)


# --------------------------------------------------------------------------
# device program
# --------------------------------------------------------------------------

def _build_program(schs):
    nc = bacc.Bacc()
    f32 = mybir.dt.float32
    bf16 = mybir.dt.bfloat16
    i16 = mybir.dt.int16

    prm = {}
    for b in (0, 1):
        sch = schs[b]
        prm[f"xT{b}"] = nc.declare_dram_parameter(f"xT{b}", [P, SH], bf16, isOutput=False)
        prm[f"idx{b}"] = nc.declare_dram_parameter(f"idx{b}", [P, sch["L"] // 16], i16, isOutput=False)
        prm[f"dl{b}"] = nc.declare_dram_parameter(f"dl{b}", [P, sch["TB"]], bf16, isOutput=False)
        prm[f"dist{b}"] = nc.declare_dram_parameter(f"dist{b}", [P, NW], f32, isOutput=False)
        prm[f"dist2{b}"] = nc.declare_dram_parameter(f"dist2{b}", [P, NW], f32, isOutput=False)
        prm[f"DtT{b}"] = nc.declare_dram_parameter(f"DtT{b}", [P, NW * G], bf16, isOutput=False)
        prm[f"ic{b}"] = nc.declare_dram_parameter(f"ic{b}", [G, 1], f32, isOutput=False)
    for nm, shp, dt in (
        ("W1eo", [P, 32], bf16), ("W1pi", [P, 32], bf16), ("b1rep", [P, 32], f32),
        ("W2p", [32, 16], bf16), ("b2rep", [P, 16], f32),
        ("W3", [16, 8], f32), ("b3r", [G, 8], f32),
        ("mW1", [16, 8], f32), ("mb1r", [G, 8], f32),
        ("mW2", [8, 2], f32), ("mb2r", [G, 2], f32),
        ("identf", [P, P], f32), ("iota", [P, P], bf16),
    ):
        prm[nm] = nc.declare_dram_parameter(nm, shp, dt, isOutput=False)
    out_p = nc.declare_dram_parameter("out", [G, 2], f32, isOutput=True)

    t1loc = [nc.dram_tensor(f"t1loc{b}", [P, 2 * CH], bf16) for b in (0, 1)]
    t1full = [nc.dram_tensor(f"t1full{b}", [NCORE * P, 2 * CH], bf16) for b in (0, 1)]
    t2loc = [nc.dram_tensor(f"t2loc{b}", [P, CH], f32) for b in (0, 1)]
    t2full = [nc.dram_tensor(f"t2full{b}", [NCORE * P, CH], f32) for b in (0, 1)]
    pool_in = nc.dram_tensor("pool_in", [G, 32], f32)
    pool_out = nc.dram_tensor("pool_out", [G, 32], f32)

    with tile.TileContext(nc) as tc:
        with (
            tc.tile_pool(name="const", bufs=1) as cp,
            tc.tile_pool(name="tabs", bufs=1) as tbp,
            tc.tile_pool(name="stream", bufs=2) as sp,
            tc.tile_pool(name="small", bufs=3) as sm,
            tc.tile_pool(name="auxp", bufs=1, space="PSUM") as auxp,
        ):
            ct = {}
            for nm in ("W1eo", "W1pi", "b1rep", "W2p", "b2rep", "W3", "b3r",
                       "mW1", "mb1r", "mW2", "mb2r", "identf", "iota"):
                t = cp.tile(list(prm[nm].shape), prm[nm].dtype, tag=nm, name=f"c_{nm}")
                nc.sync.dma_start(out=t[(slice(None),) * 2], in_=prm[nm][:, :])
                ct[nm] = t
            identb = cp.tile([P, P], bf16)
            nc.vector.tensor_copy(out=identb[:, :], in_=ct["identf"][:, :])

            dist_t, dist2_t = [], []
            for b in range(2):
                d1 = cp.tile([P, NW], f32, tag=f"dist{b}", name=f"dist_t{b}")
                nc.sync.dma_start(out=d1[:, :], in_=prm[f"dist{b}"][:, :])
                dist_t.append(d1)
                d2 = cp.tile([P, NW], f32, tag=f"dist2{b}", name=f"dist2_t{b}")
                nc.sync.dma_start(out=d2[:, :], in_=prm[f"dist2{b}"][:, :])
                dist2_t.append(d2)

            # =========== phase A: L1 table builds (both branches) ===========
            midp_ctx = tc.tile_pool(name="midp", bufs=1)
            midp = midp_ctx.__enter__()
            act1 = [midp.tile([P, NW * 32], bf16, tag=f"act1{b}", name=f"act1_{b}")
                    for b in range(2)]
            hself1 = [midp.tile([P, NW * 32], bf16, tag=f"hs1{b}", name=f"hself1_{b}")
                      for b in range(2)]
            hself2 = [None, None]
            act2 = [None, None]

            xtp_ctx = tc.tile_pool(name="xtp", bufs=1)
            xtp = xtp_ctx.__enter__()
            for b in range(2):
                xT = xtp.tile([P, CH, 8], bf16, tag="xT", name=f"xT_{b}")
                nc.sync.dma_start(out=xT[:, :, :], in_=prm[f"xT{b}"][:, :])
                hs1f = sm.tile([P, 32], f32, tag="hs1f")
                for w in range(NW):
                    hp = auxp.tile([P, 32], f32, tag="aux", space="PSUM")
                    nc.tensor.matmul(out=hp[:, :], lhsT=xT[:, 16 * w:16 * (w + 1), :],
                                     rhs=ct["W1pi"][:, :], start=True, stop=True)
                    nc.vector.tensor_scalar_mul(out=hs1f[:, :], in0=hp[:, :],
                                                scalar1=dist_t[b][:, w:w + 1])
                    nc.vector.tensor_add(
                        out=hself1[b][:, w * 32:(w + 1) * 32], in0=hs1f[:, :],
                        in1=ct["b1rep"][:, :])
                bnd = midp.tile([16, CH, 2], bf16, tag="bnd", name=f"bnd_{b}")
                NCH = 8
                cw = CH // NCH
                for g in range(8):
                    for u in range(2):
                        for chk in range(NCH):
                            c0 = chk * cw
                            bp = auxp.tile([16, cw], f32, tag="aux", space="PSUM")
                            nc.tensor.matmul(
                                out=bp[:, :],
                                lhsT=ct["W1eo"][:, 16 * u:16 * u + 16],
                                rhs=xT[:, c0:c0 + cw, g],
                                start=True, stop=True)
                            nc.vector.tensor_copy(
                                out=bnd[:, c0:c0 + cw, u], in_=bp[:, :])
                    nc.sync.dma_start(out=t1loc[b][16 * g:16 * g + 16, :],
                                      in_=bnd[:, :, :])
                nc.gpsimd.collective_compute(
                    "AllGather", mybir.AluOpType.bypass,
                    replica_groups=[list(range(NCORE))],
                    ins=[t1loc[b][:, :]], outs=[t1full[b][:, :]])
            xtp_ctx.__exit__(None, None, None)

            # =========== gather/scatter machinery ===========
            def gather_layer(b, layer, tab, aggp, tpool, hself, act_out,
                             scale_out):
                sch = schs[b]
                L, NB, sched = sch["L"], sch["NB"], sch["sched"]
                wdiv, wmul = (16, 32) if layer == 1 else (32, 16)
                first_gen, last_win = {}, {}
                for t in range(NB * 8):
                    for (col, w) in sched[t]:
                        gen = w // wdiv
                        if gen not in first_gen:
                            first_gen[gen] = col
                        last_win[w] = col
                ncall = (L + KI - 1) // KI
                oh_state = {"c0": -99999, "tile": None}
                gen_tiles = {}

                def post_window(w):
                    gen = w // wdiv
                    off = (w % wdiv) * wmul
                    ag = gen_tiles[gen]
                    tmp = sm.tile([P, 32], f32, tag="post", name=f"post_{b}_{layer}_{w}")
                    nc.vector.tensor_scalar_mul(
                        out=tmp[:, 0:wmul], in0=ag[:, off:off + wmul],
                        scalar1=dist_t[b][:, w:w + 1])
                    nc.vector.tensor_add(out=tmp[:, 0:wmul], in0=tmp[:, 0:wmul],
                                         in1=hself[:, w * wmul:(w + 1) * wmul])
                    nc.vector.tensor_scalar_max(
                        out=tmp[:, 0:wmul], in0=tmp[:, 0:wmul], scalar1=0.0)
                    if scale_out:
                        nc.vector.tensor_scalar_mul(
                            out=act_out[:, w * wmul:(w + 1) * wmul],
                            in0=tmp[:, 0:wmul], scalar1=dist_t[b][:, w:w + 1])
                    else:
                        nc.vector.tensor_copy(
                            out=act_out[:, w * wmul:(w + 1) * wmul],
                            in_=tmp[:, 0:wmul])

                for ci in range(ncall):
                    ni = min(KI, L - ci * KI)
                    idc = sp.tile([P, KI // 16], i16, tag="idc",
                                  name=f"idc_{b}_{layer}_{ci}")
                    nc.sync.dma_start(
                        out=idc[:, 0:ni // 16],
                        in_=prm[f"idx{b}"][:, ci * (KI // 16):ci * (KI // 16) + ni // 16])
                    if layer == 1:
                        msgs = sp.tile([P, KI, 2], bf16, tag="msgs",
                                       name=f"msgs1_{b}_{ci}")
                        nc.gpsimd.ap_gather(
                            out_ap=msgs[:, 0:ni, :], in_ap=tab[:, :, :],
                            idxs_ap=idc[:, 0:ni // 16],
                            channels=P, num_elems=NT, d=2, num_idxs=ni)
                    else:
                        msgs = sp.tile([P, KI], f32, tag="msgs",
                                       name=f"msgs2_{b}_{ci}")
                        nc.gpsimd.ap_gather(
                            out_ap=msgs[:, 0:ni], in_ap=tab[:, :],
                            idxs_ap=idc[:, 0:ni // 16],
                            channels=P, num_elems=NT, d=1, num_idxs=ni)
                    for kk in range(ni // P):
                        tbase = (ci * (KI // P) + kk) * 8
                        if layer == 1:
                            tp = tpool.tile([P, 2, P], bf16, tag="tp", space="PSUM")
                            for u in range(2):
                                nc.tensor.transpose(
                                    out=tp[:, u, :],
                                    in_=msgs[:, kk * P:(kk + 1) * P, u],
                                    identity=identb[:, :])
                            rhsT = sm.tile([P, 2, P], bf16, tag="rhsT")
                            nc.vector.tensor_copy(out=rhsT[:, :, :], in_=tp[:, :, :])
                        else:
                            tp = tpool.tile([P, P], f32, tag="tp", space="PSUM")
                            nc.tensor.transpose(
                                out=tp[:, :], in_=msgs[:, kk * P:(kk + 1) * P],
                                identity=ct["identf"][:, :])
                            rhsT = sm.tile([P, 2, P], bf16, tag="rhsT")
                            nc.vector.tensor_copy(out=rhsT[:, 0, :], in_=tp[:, :])
                        done_wins = []
                        for q in range(8):
                            for (col, w) in sched[tbase + q]:
                                if not (oh_state["c0"] <= col < oh_state["c0"] + 32):
                                    c0 = (col // 32) * 32
                                    dlch = sp.tile([P, 32], bf16, tag="dlch",
                                                   name=f"dlch_{b}_{layer}_{c0}")
                                    nc.sync.dma_start(
                                        out=dlch[:, :],
                                        in_=prm[f"dl{b}"][:, c0:c0 + 32])
                                    oh = sp.tile([P, 32, P], bf16, tag="oh",
                                                 name=f"oh_{b}_{layer}_{c0}")
                                    nc.vector.tensor_tensor(
                                        out=oh[:, :, :],
                                        in0=dlch[:, :, None].to_broadcast([P, 32, P]),
                                        in1=ct["iota"][:, None, :].to_broadcast([P, 32, P]),
                                        op=mybir.AluOpType.is_equal)
                                    oh_state["c0"] = c0
                                    oh_state["tile"] = oh
                                ohc = oh_state["tile"]
                                gen = w // wdiv
                                if gen not in gen_tiles:
                                    gen_tiles[gen] = aggp.tile(
                                        [P, 512], f32, tag=f"agg{gen % 4}",
                                        name=f"agg_{b}_{layer}_{gen}", space="PSUM")
                                ag = gen_tiles[gen]
                                off = (w % wdiv) * wmul
                                st = (first_gen[gen] == col)
                                if layer == 1:
                                    nc.tensor.matmul(
                                        out=ag[:, off:off + 16],
                                        lhsT=ohc[:, col - oh_state["c0"], :],
                                        rhs=rhsT[:, 0, 16 * q:16 * q + 16],
                                        start=st, stop=False,
                                        skip_group_check=True)
                                    nc.tensor.matmul(
                                        out=ag[:, off + 16:off + 32],
                                        lhsT=ohc[:, col - oh_state["c0"], :],
                                        rhs=rhsT[:, 1, 16 * q:16 * q + 16],
                                        start=False, stop=(last_win[w] == col),
                                        skip_group_check=True)
                                else:
                                    nc.tensor.matmul(
                                        out=ag[:, off:off + 16],
                                        lhsT=ohc[:, col - oh_state["c0"], :],
                                        rhs=rhsT[:, 0, 16 * q:16 * q + 16],
                                        start=st, stop=(last_win[w] == col),
                                        skip_group_check=True)
                                if last_win[w] == col:
                                    done_wins.append(w)
                        for w in done_wins:
                            post_window(w)

            def build_tab2(b, latep):
                act1T = midp.tile([32, CH, 8], bf16, tag="act1T", name=f"act1T_{b}")
                for w in range(NW):
                    ap_ = auxp.tile([32, P], bf16, tag="aux", space="PSUM")
                    nc.tensor.transpose(out=ap_[:, :],
                                        in_=act1[b][:, w * 32:(w + 1) * 32],
                                        identity=identb[:, :])
                    nc.vector.tensor_copy(out=act1T[:, 16 * w:16 * (w + 1), :],
                                          in_=ap_[:, :])
                hs2 = latep.tile([P, NW * 16], bf16, tag=f"hs2_{b}", name=f"hself2_{b}")
                hs2f = sm.tile([P, 16], f32, tag="hs2f")
                for w in range(NW):
                    hp = auxp.tile([P, 16], f32, tag="aux", space="PSUM")
                    nc.tensor.matmul(out=hp[:, :],
                                     lhsT=act1T[:, 16 * w:16 * (w + 1), :],
                                     rhs=ct["W2p"][:, :], start=True, stop=True)
                    nc.vector.tensor_scalar_mul(out=hs2f[:, :], in0=hp[:, :],
                                                scalar1=dist_t[b][:, w:w + 1])
                    nc.vector.tensor_add(
                        out=hs2[:, w * 16:(w + 1) * 16], in0=hs2f[:, :],
                        in1=ct["b2rep"][:, :])
                bnd2 = midp.tile([16, CH], f32, tag="bnd2", name=f"bnd2_{b}")
                NCH = 8
                cw = CH // NCH
                for g in range(8):
                    for chk in range(NCH):
                        c0 = chk * cw
                        bp = auxp.tile([16, cw], f32, tag="aux", space="PSUM")
                        nc.tensor.matmul(
                            out=bp[:, :], lhsT=ct["W2p"][:, :],
                            rhs=act1T[:, c0:c0 + cw, g],
                            start=True, stop=True)
                        nc.vector.tensor_copy(
                            out=bnd2[:, c0:c0 + cw], in_=bp[:, :])
                    nc.sync.dma_start(out=t2loc[b][16 * g:16 * g + 16, :],
                                      in_=bnd2[:, :])
                nc.gpsimd.collective_compute(
                    "AllGather", mybir.AluOpType.bypass,
                    replica_groups=[list(range(NCORE))],
                    ins=[t2loc[b][:, :]], outs=[t2full[b][:, :]])
                return hs2

            # =========== phase B: L1 gathers + L2 table builds ===========
            latep_ctx = tc.tile_pool(name="latep", bufs=1)
            latep = latep_ctx.__enter__()

            l1p_ctx = tc.tile_pool(name="l1p", bufs=1, space="PSUM")
            l1p = l1p_ctx.__enter__()
            tp1_ctx = tc.tile_pool(name="tp1p", bufs=2, space="PSUM")
            tp1p = tp1_ctx.__enter__()
            for b in range(2):
                tab1 = tbp.tile([P, NT, 2], bf16, tag="tabfull", name=f"tab1_{b}")
                for c in range(NCORE):
                    nc.sync.dma_start(out=tab1[:, c * CH:(c + 1) * CH, :],
                                      in_=t1full[b][c * P:(c + 1) * P, :])
                gather_layer(b, 1, tab1, l1p, tp1p, hself1[b], act1[b],
                             scale_out=True)
                hself2[b] = build_tab2(b, latep)
            tp1_ctx.__exit__(None, None, None)
            l1p_ctx.__exit__(None, None, None)

            # =========== phase C: L2 gathers + pool partials ===========
            l2p_ctx = tc.tile_pool(name="l2p", bufs=1, space="PSUM")
            l2p = l2p_ctx.__enter__()
            tp2_ctx = tc.tile_pool(name="tp2p", bufs=2, space="PSUM")
            tp2p = tp2_ctx.__enter__()
            for b in range(2):
                tab2 = tbp.tile([P, NT], f32, tag="tabfull", name=f"tab2_{b}")
                for c in range(NCORE):
                    nc.sync.dma_start(out=tab2[:, c * CH:(c + 1) * CH],
                                      in_=t2full[b][c * P:(c + 1) * P, :])
                a2 = latep.tile([P, NW * 16], bf16, tag=f"act2{b}", name=f"act2_{b}")
                act2[b] = a2
                gather_layer(b, 2, tab2, l2p, tp2p, hself2[b], a2,
                             scale_out=False)
                DtT = latep.tile([P, NW * G], bf16, tag="DtT", name=f"DtT_{b}")
                nc.sync.dma_start(out=DtT[:, :], in_=prm[f"DtT{b}"][:, :])
                pp = auxp.tile([G, 16], f32, tag="aux", name=f"poolp_{b}",
                               space="PSUM")
                for w in range(NW):
                    nc.tensor.matmul(out=pp[:, :],
                                     lhsT=DtT[:, w * G:(w + 1) * G],
                                     rhs=a2[:, w * 16:(w + 1) * 16],
                                     start=(w == 0), stop=(w == NW - 1))
                ps = sm.tile([G, 16], f32, tag="pools")
                nc.vector.tensor_copy(out=ps[:, :], in_=pp[:, :])
                nc.sync.dma_start(out=pool_in[:, 16 * b:16 * (b + 1)], in_=ps[:, :])
            tp2_ctx.__exit__(None, None, None)
            l2p_ctx.__exit__(None, None, None)

            # =========== tail: AllReduce + pool scale + W3 + MLP ===========
            nc.gpsimd.collective_compute(
                "AllReduce", mybir.AluOpType.add,
                replica_groups=[list(range(NCORE))],
                ins=[pool_in[:, :]], outs=[pool_out[:, :]])
            pr = sm.tile([G, 32], f32, tag="pr")
            nc.sync.dma_start(out=pr[:, :], in_=pool_out[:, :])
            pm_ctx = tc.tile_pool(name="pm", bufs=1, space="PSUM")
            pm = pm_ctx.__enter__()
            pooled_cat = sm.tile([G, 16], f32, tag="pcat")
            for b in range(2):
                ic_t = sm.tile([G, 1], f32, tag="ic")
                nc.sync.dma_start(out=ic_t[:, :], in_=prm[f"ic{b}"][:, :])
                pb = sm.tile([G, 16], f32, tag="pb")
                nc.vector.tensor_scalar_mul(out=pb[:, :], in0=pr[:, 16 * b:16 * (b + 1)],
                                            scalar1=ic_t[:, :])
                pbT_p = pm.tile([16, G], f32, tag="pbT", name=f"pbT_{b}", space="PSUM")
                nc.tensor.transpose(out=pbT_p[:, :], in_=pb[:, :],
                                    identity=ct["identf"][0:G, 0:G])
                pbT = sm.tile([16, G], f32, tag="pbTs")
                nc.vector.tensor_copy(out=pbT[:, :], in_=pbT_p[:, :])
                po_p = pm.tile([G, 8], f32, tag="po", name=f"po_{b}", space="PSUM")
                nc.tensor.matmul(out=po_p[:, :], lhsT=pbT[:, :], rhs=ct["W3"][:, :],
                                 start=True, stop=True)
                nc.vector.tensor_add(out=pooled_cat[:, 8 * b:8 * (b + 1)],
                                     in0=po_p[:, :], in1=ct["b3r"][:, :])
            pcT_p = pm.tile([16, G], f32, tag="pcT", space="PSUM")
            nc.tensor.transpose(out=pcT_p[:, :], in_=pooled_cat[:, :],
                                identity=ct["identf"][0:G, 0:G])
            pcT = sm.tile([16, G], f32, tag="pcTs")
            nc.vector.tensor_copy(out=pcT[:, :], in_=pcT_p[:, :])
            m1_p = pm.tile([G, 8], f32, tag="m1", space="PSUM")
            nc.tensor.matmul(out=m1_p[:, :], lhsT=pcT[:, :], rhs=ct["mW1"][:, :],
                             start=True, stop=True)
            m1_s = sm.tile([G, 8], f32, tag="m1s")
            nc.vector.tensor_add(out=m1_s[:, :], in0=m1_p[:, :], in1=ct["mb1r"][:, :])
            nc.vector.tensor_scalar_max(out=m1_s[:, :], in0=m1_s[:, :], scalar1=0.0)
            m1T_p = pm.tile([8, G], f32, tag="m1T", space="PSUM")
            nc.tensor.transpose(out=m1T_p[:, :], in_=m1_s[:, :],
                                identity=ct["identf"][0:G, 0:G])
            m1T = sm.tile([8, G], f32, tag="m1Ts")
            nc.vector.tensor_copy(out=m1T[:, :], in_=m1T_p[:, :])
            m2_p = pm.tile([G, 2], f32, tag="m2", space="PSUM")
            nc.tensor.matmul(out=m2_p[:, :], lhsT=m1T[:, :], rhs=ct["mW2"][:, :],
                             start=True, stop=True)
            m2_s = sm.tile([G, 2], f32, tag="m2s")
            nc.vector.tensor_add(out=m2_s[:, :], in0=m2_p[:, :], in1=ct["mb2r"][:, :])
            nc.sync.dma_start(out=out_p[:, :], in_=m2_s[:, :])
            pm_ctx.__exit__(None, None, None)

            latep_ctx.__exit__(None, None, None)
            midp_ctx.__exit__(None, None, None)

    nc.compile()
    return nc


# --------------------------------------------------------------------------
# driver
# --------------------------------------------------------------------------

def _run(inputs, trace=False):
    global last_results
    x = [np.asarray(inputs["x0"], np.float32), np.asarray(inputs["x1"], np.float32)]
    ei = [np.asarray(inputs["edge_index0"]), np.asarray(inputs["edge_index1"])]
    bt = [np.asarray(inputs["batch0"]), np.asarray(inputs["batch1"])]

    schs = [_prep_branch(ei[b], bt[b]) for b in range(2)]

    W1 = np.asarray(inputs["W1"], np.float32)
    b1 = np.asarray(inputs["b1"], np.float32)
    W2 = np.asarray(inputs["W2"], np.float32)
    b2 = np.asarray(inputs["b2"], np.float32)
    W1eo = np.concatenate([W1[:, 0::2], W1[:, 1::2]], axis=1).astype(bfloat16)
    common = dict(
        W1eo=W1eo,
        W1pi=W1[:, PI].astype(bfloat16),
        b1rep=np.broadcast_to(b1[PI], (P, 32)).astype(np.float32).copy(),
        W2p=np.asarray(W2[PI, :], np.float32).astype(bfloat16),
        b2rep=np.broadcast_to(b2, (P, 16)).astype(np.float32).copy(),
        W3=np.asarray(inputs["W3"], np.float32),
        b3r=np.broadcast_to(np.asarray(inputs["b3"], np.float32), (G, 8)).copy(),
        mW1=np.asarray(inputs["mW1"], np.float32),
        mb1r=np.broadcast_to(np.asarray(inputs["mb1"], np.float32), (G, 8)).copy(),
        mW2=np.asarray(inputs["mW2"], np.float32),
        mb2r=np.broadcast_to(np.asarray(inputs["mb2"], np.float32), (G, 2)).copy(),
        identf=np.eye(P, dtype=np.float32),
        iota=np.ascontiguousarray(
            np.broadcast_to(np.arange(P, dtype=np.float32), (P, P))).astype(bfloat16),
        ic0=schs[0]["inv_cnt"], ic1=schs[1]["inv_cnt"],
    )

    xpad = []
    diss = []
    for b in range(2):
        t = np.zeros((NPAD, 128), np.float32)
        t[:N] = x[b]
        xpad.append(t)
        diss.append(schs[b]["dis"])

    in_maps = []
    for c in range(NCORE):
        m = dict(common)
        for b in range(2):
            pc = schs[b]["per_core"][c]
            m[f"xT{b}"] = np.ascontiguousarray(
                (xpad[b][c * SH:(c + 1) * SH]
                 * diss[b][c * SH:(c + 1) * SH, None]).T).astype(bfloat16)
            m[f"idx{b}"] = pc["idx"]
            m[f"dl{b}"] = pc["dl"]
            m[f"dist{b}"] = pc["dist"]
            m[f"dist2{b}"] = pc["dist2"]
            m[f"DtT{b}"] = pc["DtT"]
        in_maps.append(m)

    nc = _build_program(schs)
    res = run_bass_kernel_spmd(nc, in_maps, list(range(NCORE)), trace=trace)
    last_results = res
    return np.asarray(res.results[0]["out"], np.float32)


def kernel(**inputs):
    return _run(inputs, trace=False)
